# revision 21
# baseline (speedup 1.0000x reference)
"""Trainium2 Bass kernel for nn_ApproxCompressor (v5).

Reference (per sample n):
    alpha = sigmoid(z_alpha); h[k] = (1-alpha)*alpha^k (k<16384)
    env   = causal_conv(mean_c x^2, h); LG = log(env + 1e-5)
    quadratic-knee gain; out = gain * x.

Kernel strategy (8 cores x 4 samples, pure data parallel):
  * Time-on-partition layout: sample time t = 128*j + p (partition p,
    column j; 1024 columns per sample).  The exponential FIR truncated at
    256 taps (a^256 < 1e-7, asserted on host) is then EXACTLY a 2-tap
    block convolution over columns:
        Y[:, j] = W0 @ D[:, j] + W1 @ D[:, j-1]
    with W0[q,p] = a^(p-q) (p>=q), W1[q,p] = a^(128+p-q).  Both terms run
    as bf16 matmuls on the otherwise-idle PE, accumulating in PSUM; the
    per-channel energy add is folded in by matmul'ing each channel's
    squares separately (4 matmuls/sample).  This replaces the serial DVE
    scan of v4 entirely.
  * Knee, branch-free:  A = relu(LG + W - T), B = relu(A - 2W) (= relu
    with shifted bias), C = min(A, 2W) = A - B, Z = A + B = 2A - C,
    log_gain = c/(4W) * C * Z.  Ops are spread across DVE (4x/2x bf16
    perf modes), ACT (Ln / relu-B / Exp with per-partition scale+bias)
    and Pool (C = tensor_scalar min), chosen to balance engine time.
  * All elementwise tensors are bf16 (packed, SBUF) for DVE 2x/4x modes;
    PSUM accumulation and scalar params stay f32.
  * I/O: host ships bf16 device-layout input (time-on-partition); output
    written bf16 device layout, unshuffled on host.  DMAs spread across
    all three DGE queues (sync + act HWDGE, pool SWDGE) in per-sample
    chunks so compute starts after the first sample lands.
"""

import os
import sys

import numpy as np


def _import_concourse():
    try:
        import concourse.bass  # noqa: F401
    except ImportError:
        for p in ("/opt/trn_rl_repo", "/root/.axon_site/_ro/trn_rl_repo"):
            if os.path.isdir(p) and p not in sys.path:
                sys.path.insert(0, p)
        import concourse.bass  # noqa: F401


_import_concourse()

import ml_dtypes  # noqa: E402
import concourse.bass as bass  # noqa: E402
import concourse.tile as tile  # noqa: E402
from concourse import bacc, mybir  # noqa: E402

N, C, L = 32, 2, 131072
NCORES = 8
NLOC = N // NCORES  # 4 samples/core
P = 128
COLS = L // P  # 1024 columns per sample
SROW = C * COLS  # 2048 elems per sample per partition row
ROW = NLOC * SROW  # 8192 elems per partition row
EPS = 1e-5
K_FIR = 16384
JF = 256  # FIR truncation (2 blocks of 128)

F32 = mybir.dt.float32
BF16 = mybir.dt.bfloat16

# per-sample param column slots (prm tile is [P, NLOC*NPRM] f32)
PRM_LNSCALE, PRM_EPS, PRM_B1, PRM_B1M2W, PRM_W2, PRM_C4W = 0, 1, 2, 3, 4, 5
NPRM = 8
ACT_SET_ID = 6  # natural_log_exp_and_others: ln, exp, relu, square

TRACE_RESULT = {}


def build_nc():
    AF = mybir.ActivationFunctionType
    OP = mybir.AluOpType

    nc = bacc.Bacc("TRN2", target_bir_lowering=False, num_devices=NCORES)
    xd_ext = nc.declare_dram_parameter("xd", [P, ROW], BF16, isOutput=False)
    prm_ext = nc.declare_dram_parameter("prm", [P, NLOC * NPRM], F32, isOutput=False)
    wm_ext = nc.declare_dram_parameter("wm", [P, NLOC * 2 * P], BF16, isOutput=False)
    od_ext = nc.declare_dram_parameter("od", [P, ROW], BF16, isOutput=True)

    with tile.TileContext(nc) as tc:
        atl = mybir.InstLoadActFuncSet(
            name=nc.get_next_instruction_name(), ins=[], outs=[],
            act_func_set_id=ACT_SET_ID,
        )
        nc.scalar.add_instruction(atl)
        with (
            tc.tile_pool(name="pc", bufs=1) as pc,
            tc.tile_pool(name="pin", bufs=NLOC) as pin,
            tc.tile_pool(name="psq", bufs=4) as psq,
            tc.tile_pool(name="pk", bufs=3) as pk,
            tc.tile_pool(name="po", bufs=3) as po,
            tc.tile_pool(name="pps", bufs=1, space=bass.MemorySpace.PSUM) as pps,
        ):
            # ---- input layout across the 3 DGE queues ----------------------
            # s0 halves ride both HWDGE queues (sync+act) to land first;
            # wm goes on the pool SWDGE queue (its ~3us startup still beats
            # the first matmul's need time); pool only issues EARLY DMAs so
            # its end-of-kernel drain overlaps compute.
            wm = pc.tile([P, NLOC * 2 * P], BF16, tag="wm")
            prm = pc.tile([P, NLOC * NPRM], F32, tag="prm")

            def col(s, k):
                return prm[:, s * NPRM + k : s * NPRM + k + 1]

            xt = [pin.tile([P, SROW], BF16, tag="x", name=f"xs{s}")
                  for s in range(NLOC)]

            def ld(xs, s, half, eng):
                rows = slice(0, 64) if half == 0 else slice(64, 128)
                eng.dma_start(out=xs[rows, :],
                              in_=xd_ext[rows, s * SROW : (s + 1) * SROW])

            ld(xt[0], 0, 0, nc.sync)
            ld(xt[0], 0, 1, nc.scalar)
            ld(xt[1], 1, 1, nc.gpsimd)
            ld(xt[1], 1, 0, nc.sync)
            nc.scalar.dma_start(out=wm[:], in_=wm_ext[:])
            nc.scalar.dma_start(out=prm[:], in_=prm_ext[:])
            ld(xt[2], 2, 0, nc.sync)
            ld(xt[2], 2, 1, nc.gpsimd)
            ld(xt[3], 3, 0, nc.sync)
            ld(xt[3], 3, 1, nc.gpsimd)

            sqs, ys, lgs, gs, ods, Qs = {}, {}, {}, {}, {}, {}
            HB = COLS // 2  # psum bank = 512 f32 columns

            def emit_sq(s):
                sq = psq.tile([P, SROW], BF16, tag="sq")
                nc.vector.tensor_tensor(sq[:], xt[s][:], xt[s][:], OP.mult)
                sqs[s] = sq

            def mm_bank(yt, w0, w1, sq, bank):
                o0 = bank * HB
                lo = 1 if bank == 0 else 0
                for cch in range(C):
                    o = cch * COLS + o0
                    nc.tensor.matmul(yt[:, 0:HB], w0, sq[:, o : o + HB],
                                     start=cch == 0, stop=False)
                for cch in range(C):
                    o = cch * COLS + o0
                    nc.tensor.matmul(yt[:, lo:HB], w1,
                                     sq[:, o + lo - 1 : o + HB - 1],
                                     start=False, stop=cch == C - 1)

            def emit_mm(s):
                w0 = wm[:, s * 2 * P : s * 2 * P + P]
                w1 = wm[:, s * 2 * P + P : s * 2 * P + 2 * P]
                sq = sqs[s]
                if s == 0:
                    # bank-granular psum tiles: Ln0a starts after 4 matmuls
                    ya = pps.tile([P, HB], F32, tag="ya")
                    yb = pps.tile([P, HB], F32, tag="yb")
                    mm_bank(ya, w0, w1, sq, 0)
                    mm_bank(yb, w0, w1, sq, 1)
                    ys[s] = (ya, yb)
                else:
                    y = pps.tile([P, COLS], F32, tag="y", bufs=3)
                    for cch in range(C):
                        o = cch * COLS
                        st = cch == 0
                        nc.tensor.matmul(y[:, 0:HB], w0, sq[:, o : o + HB],
                                         start=st, stop=False)
                        nc.tensor.matmul(y[:, HB:COLS], w0,
                                         sq[:, o + HB : o + COLS],
                                         start=st, stop=False)
                    for cch in range(C):
                        o = cch * COLS
                        sp = cch == C - 1
                        nc.tensor.matmul(y[:, 1:HB], w1, sq[:, o : o + HB - 1],
                                         start=False, stop=sp)
                        nc.tensor.matmul(y[:, HB:COLS], w1,
                                         sq[:, o + HB - 1 : o + COLS - 1],
                                         start=False, stop=sp)
                    ys[s] = y

            def emit_ln(s):
                lg = pk.tile([P, COLS], BF16, tag="lg")
                if s == 0:
                    ya, yb = ys[s]
                    nc.scalar.activation(lg[:, 0:HB], ya[:], AF.Ln,
                                         bias=col(s, PRM_EPS),
                                         scale=col(s, PRM_LNSCALE))
                    nc.scalar.activation(lg[:, HB:COLS], yb[:], AF.Ln,
                                         bias=col(s, PRM_EPS),
                                         scale=col(s, PRM_LNSCALE))
                else:
                    nc.scalar.activation(lg[:], ys[s][:], AF.Ln,
                                         bias=col(s, PRM_EPS),
                                         scale=col(s, PRM_LNSCALE))
                lgs[s] = lg

            def emit_knee(s, a_on_act):
                lg = lgs[s]
                A = pk.tile([P, COLS], BF16, tag="A")
                if a_on_act:
                    nc.scalar.activation(A[:], lg[:], AF.Relu,
                                         bias=col(s, PRM_B1))
                else:
                    nc.vector.tensor_scalar(A[:], lg[:], col(s, PRM_B1), 0.0,
                                            OP.add, OP.max)
                B = pk.tile([P, COLS], BF16, tag="B")
                nc.scalar.activation(B[:], lg[:], AF.Relu,
                                     bias=col(s, PRM_B1M2W))
                Ct = pk.tile([P, COLS], BF16, tag="C")
                nc.vector.tensor_scalar(Ct[:], A[:], col(s, PRM_W2), None,
                                        OP.min)
                Z = pk.tile([P, COLS], BF16, tag="Z")
                nc.vector.tensor_tensor(Z[:], A[:], B[:], OP.add)
                Q = pk.tile([P, COLS], BF16, tag="Q")
                nc.vector.tensor_tensor(Q[:], Ct[:], Z[:], OP.mult)
                Qs[s] = Q

            def emit_exp(s, c0=0, c1=COLS):
                if s not in gs:
                    gs[s] = pk.tile([P, COLS], BF16, tag="g", name=f"g{s}")
                g = gs[s]
                nc.scalar.activation(g[:, c0:c1], Qs[s][:, c0:c1], AF.Exp,
                                     scale=col(s, PRM_C4W))

            def emit_out(s, c0=0, c1=COLS):
                if s not in ods:
                    ods[s] = po.tile([P, SROW], BF16, tag="od", name=f"od{s}")
                od = ods[s]
                w = c1 - c0
                o0 = od[:, c0 : c0 + w]
                o3 = bass.AP(o0.tensor, o0.offset,
                             [list(o0.ap[0]), [COLS, C], [1, w]])
                x0 = xt[s][:, c0 : c0 + w]
                x3 = bass.AP(x0.tensor, x0.offset,
                             [list(x0.ap[0]), [COLS, C], [1, w]])
                g0 = gs[s][:, c0 : c0 + w]
                g3 = bass.AP(g0.tensor, g0.offset,
                             [list(g0.ap[0]), [0, C], [1, w]])
                nc.vector.tensor_tensor(o3, g3, x3, OP.mult)

            def emit_store(s, eng_half0, eng_half1=None):
                od = ods[s]
                dsl = slice(s * SROW, (s + 1) * SROW)
                if eng_half1 is None:
                    eng_half0.dma_start(out=od_ext[:, dsl], in_=od[:])
                else:
                    eng_half0.dma_start(out=od_ext[0:64, dsl], in_=od[0:64, :])
                    eng_half1.dma_start(out=od_ext[64:128, dsl], in_=od[64:128, :])

            # ---- pipelined emission ---------------------------------------
            emit_sq(0)
            emit_mm(0)
            emit_sq(1)
            emit_ln(0)
            emit_mm(1)
            emit_knee(0, a_on_act=False)
            emit_sq(2)
            emit_ln(1)
            emit_exp(0)
            emit_mm(2)
            emit_out(0)
            emit_knee(1, a_on_act=True)
            emit_store(0, nc.sync)
            emit_sq(3)
            emit_ln(2)
            emit_exp(1)
            emit_mm(3)
            emit_out(1)
            emit_knee(2, a_on_act=False)
            emit_store(1, nc.sync)
            emit_ln(3)
            emit_exp(2)
            emit_out(2)
            emit_knee(3, a_on_act=False)
            emit_store(2, nc.sync)
            emit_exp(3, 0, HB)
            emit_out(3, 0, HB)
            emit_exp(3, HB, COLS)
            emit_out(3, HB, COLS)
            emit_store(3, nc.scalar, nc.sync)

    nc.finalize()
    return nc


def host_params(z_alpha, log_threshold, log_ratio, log_knee):
    z = z_alpha.astype(np.float64).reshape(-1)
    alpha = 1.0 / (1.0 + np.exp(-z))
    aK = np.exp(K_FIR * np.log(alpha))
    assert np.all(aK < 1e-6), "FIR tail non-negligible; needs shift correction"
    aJ = np.exp(JF * np.log(alpha))
    assert np.all(aJ < 1e-7), "block-conv truncation at 256 taps too short"
    T = log_threshold.astype(np.float64).reshape(-1) - 6.0
    R = 1.0 + np.exp(log_ratio.astype(np.float64).reshape(-1))
    W = np.exp(log_knee.astype(np.float64).reshape(-1))
    c = 1.0 / R - 1.0

    n = alpha.shape[0]
    prms, wms = [], []
    dp = np.arange(P)[:, None] - np.arange(P)[None, :]  # q - p -> use p-q
    pq = -dp  # pq[q, p] = p - q
    for c0 in range(n // NLOC):
        sl = slice(c0 * NLOC, (c0 + 1) * NLOC)
        a4, T4, W4, c4 = alpha[sl], T[sl], W[sl], c[sl]
        prm = np.zeros((P, NLOC * NPRM), np.float64)
        wm = np.zeros((P, NLOC * 2 * P), np.float64)
        for s in range(NLOC):
            o = s * NPRM
            prm[:, o + PRM_LNSCALE] = 0.5 * (1.0 - a4[s])
            prm[:, o + PRM_EPS] = EPS
            prm[:, o + PRM_B1] = W4[s] - T4[s]
            prm[:, o + PRM_B1M2W] = -W4[s] - T4[s]
            prm[:, o + PRM_W2] = 2.0 * W4[s]
            prm[:, o + PRM_C4W] = c4[s] / (4.0 * W4[s])
            la = np.log(a4[s])
            e0 = pq * la
            w0 = np.where((pq >= 0) & (e0 > -100.0), np.exp(e0), 0.0)
            e1 = (P + pq) * la
            w1 = np.where(e1 > -100.0, np.exp(e1), 0.0)
            wm[:, s * 2 * P : s * 2 * P + P] = w0
            wm[:, s * 2 * P + P : s * 2 * P + 2 * P] = w1
        prms.append(prm.astype(np.float32))
        wms.append(wm.astype(np.float32).astype(ml_dtypes.bfloat16))
    return prms, wms


def shuffle_in(x_core):
    """(NLOC, C, L) f32 -> (P, ROW) bf16 device layout (time-on-partition)."""
    xb = x_core.astype(np.float32).astype(ml_dtypes.bfloat16)
    v = xb.reshape(NLOC, C, COLS, P).transpose(3, 0, 1, 2)
    return np.ascontiguousarray(v.reshape(P, ROW))


def unshuffle_out(od):
    """(P, ROW) bf16 device layout -> (NLOC, C, L) f32."""
    v = od.reshape(P, NLOC, C, COLS).astype(np.float32)
    return v.transpose(1, 2, 3, 0).reshape(NLOC, C, L)


def _ensure_ntff_hook():
    import types

    try:
        from antenv.axon_hooks import get_axon_ntff_profile_hook  # noqa: F401

        return
    except ImportError:
        pass
    try:
        from trn_agent_boot.trn_boot import _ntff_profile_via_ctypes
    except ImportError:
        return
    hook = _ntff_profile_via_ctypes("/opt/axon/libaxon_pjrt.so")
    mod = types.ModuleType("antenv.axon_hooks")
    mod._hook = hook
    mod.get_axon_ntff_profile_hook = lambda: mod._hook

    def set_axon_ntff_profile_hook(h):
        mod._hook = h

    mod.set_axon_ntff_profile_hook = set_axon_ntff_profile_hook
    import antenv

    sys.modules["antenv.axon_hooks"] = mod
    antenv.axon_hooks = mod


def kernel(input_signals, z_alpha, log_threshold, log_ratio, log_knee):
    from concourse.bass_utils import run_bass_kernel_spmd

    x = np.asarray(input_signals, np.float32)
    prms, wms = host_params(
        np.asarray(z_alpha), np.asarray(log_threshold),
        np.asarray(log_ratio), np.asarray(log_knee),
    )

    nc = build_nc()
    core_ids = list(range(NCORES))
    in_maps = [
        {
            "xd": shuffle_in(x[i * NLOC : (i + 1) * NLOC]),
            "prm": prms[i],
            "wm": wms[i],
        }
        for i in core_ids
    ]

    trace = os.environ.get("BASS_KERNEL_TRACE", "0") == "1"
    if trace:
        _ensure_ntff_hook()
    res = run_bass_kernel_spmd(nc, in_maps, core_ids, trace=trace)
    if trace:
        TRACE_RESULT["exec_time_ns"] = res.exec_time_ns
        TRACE_RESULT["results"] = res

    out = np.empty((N, C, L), np.float32)
    for i in core_ids:
        out[i * NLOC : (i + 1) * NLOC] = unshuffle_out(
            np.asarray(res.results[i]["od"])
        )
    return out
